# revision 8
# baseline (speedup 1.0000x reference)
"""CGMM (Contextual Graph Markov Model) forward pass on 8 Trainium2 NeuronCores.

Self-contained: takes FULL inputs as numpy arrays, shards nodes/edges across
the 8 cores (graph parallel), runs a Bass/Tile kernel via
run_bass_kernel_spmd, returns the FULL [N, L, G] log-likelihood output.

Algorithm layout (per core, nodes on partitions, cg = g*8 + c on free dim):
  layer 0:  u0[n, cg] = B0[c, x_n, g]*Pi[c, g]  via one-hot(x) matmul
            Z = sum_c u, ll0 = log Z, h = u/Z  (h stored bf16, row-major)
  layers 1..3:
            all-gather h across cores  ->  h_full [N, 128] bf16 (Shared DRAM)
            gather h_full[src] per edge (dma_gather, 256B rows)
            aggr[dst, cg] = segment-sum via one-hot(dst_local) matmuls (PSUM fp32)
            cnt from row-sums of aggr (h rows sum to G exactly)
            QA = Qbig @ aggr^T (PE transpose + fp32 matmul)
            u = Bx * QA; Z = sum_c u; ll = log Z - log(cnt); h = u/Z
Edge streams are host-preprocessed: sorted by (dst block, src half), padded to
a cross-core-uniform tile schedule; padded slots gather row 0 with
dst_local = -1 (one-hot row of zeros -> no contribution).
"""
import os
import sys

sys.path.insert(0, "/opt/trn_rl_repo")

import numpy as np
import ml_dtypes

BF = ml_dtypes.bfloat16

# ---- problem sizes (hardcoded per contract) --------------------------------
N, E, C, M, G, L = 50000, 800000, 8, 32, 16, 4
NCORES = 8
CG = C * G  # 128


class Cfg:
    def __init__(self, n=N, e=E, ncores=NCORES, tg=64):
        self.n = n
        self.e = e
        self.ncores = ncores
        self.npc = n // ncores
        self.nb = (self.npc + 127) // 128
        self.half = n // 2
        self.tg = tg  # gather chunk size in 128-edge tiles


# ---- host preprocessing -----------------------------------------------------

def preprocess(x, edge_index, cfg):
    """Build per-core aux arrays + the (cross-core uniform) tile schedule."""
    dst = np.asarray(edge_index[0], dtype=np.int64)
    src = np.asarray(edge_index[1], dtype=np.int64)
    x = np.asarray(x, dtype=np.int64)
    nc_, npc, nb, half = cfg.ncores, cfg.npc, cfg.nb, cfg.half

    owner = dst // npc
    per_core = []
    cntAB = np.zeros((nc_, nb, 2), dtype=np.int64)
    for c in range(nc_):
        sel = owner == c
        d = dst[sel] - c * npc
        s = src[sel]
        b = d // 128
        order = np.argsort(b, kind="stable")
        b, d, s = b[order], d[order], s[order]
        dl = d % 128
        hf = (s >= half).astype(np.int64)
        per_core.append((b, dl, s, hf))
        # counts per (block, half)
        key = b * 2 + hf
        cnt = np.bincount(key, minlength=nb * 2).reshape(nb, 2)
        cntAB[c] = cnt
    TA = np.maximum(1, -(-cntAB[:, :, 0].max(axis=0) // 128))
    TB = np.maximum(1, -(-cntAB[:, :, 1].max(axis=0) // 128))
    totTA, totTB = int(TA.sum()), int(TB.sum())
    offA = np.concatenate([[0], np.cumsum(TA)]).astype(np.int64)  # tile offsets
    offB = np.concatenate([[0], np.cumsum(TB)]).astype(np.int64)

    cores = []
    for c in range(nc_):
        b, dl, s, hf = per_core[c]
        idxA = np.zeros(totTA * 128, dtype=np.int64)
        dlA = np.full(totTA * 128, -1, dtype=np.int64)
        idxB = np.zeros(totTB * 128, dtype=np.int64)
        dlB = np.full(totTB * 128, -1, dtype=np.int64)
        for bb in range(nb):
            mA = (b == bb) & (hf == 0)
            mB = (b == bb) & (hf == 1)
            nA, nB_ = int(mA.sum()), int(mB.sum())
            a0, b0 = offA[bb] * 128, offB[bb] * 128
            idxA[a0:a0 + nA] = s[mA]
            dlA[a0:a0 + nA] = dl[mA]
            idxB[b0:b0 + nB_] = s[mB] - half
            dlB[b0:b0 + nB_] = dl[mB]

        # idx dram layout: [128, cols] int16; index i at [i%16, i//16], the
        # 16-row block replicated 8x down the partitions (one copy per Q7 core)
        allidx = np.concatenate([idxA, idxB]).astype(np.int16)
        idx16 = allidx.reshape(-1, 16).T  # [16, tot/16]
        idx_d = np.tile(idx16, (8, 1))    # [128, tot/16]

        # dstloc dram layout: [128, T_tot] bf16, partition = slot within tile
        alldl = np.concatenate([dlA, dlB]).astype(np.float32)
        dl_d = alldl.reshape(-1, 128).T.copy()  # [128, T_tot] fp32

        # x dram layout: [128, nb], partition-major, fp32
        xloc = np.zeros(nb * 128, dtype=np.float32)
        xloc[:npc] = x[c * npc:(c + 1) * npc]
        x_d = xloc.reshape(nb, 128).T.copy()  # [128, nb]

        cores.append({"idx": np.ascontiguousarray(idx_d),
                      "dstloc": np.ascontiguousarray(dl_d),
                      "xq": np.ascontiguousarray(x_d)})
    return cores, TA.astype(int), TB.astype(int)


def permute_params(lambda_B0, lambda_Pi, lambda_Q, lambda_B):
    """Pure layout permutations (no compute): partition (g, c/k)-major views."""
    lamB0p = np.ascontiguousarray(
        np.transpose(np.asarray(lambda_B0, np.float32), (2, 0, 1)).reshape(G * C, M))
    lamPip = np.ascontiguousarray(np.asarray(lambda_Pi, np.float32).T)  # [G, C]
    lamQp = np.ascontiguousarray(
        np.transpose(np.asarray(lambda_Q, np.float32), (0, 3, 2, 1)).reshape(
            L - 1, G * C, C))
    lamBp = np.ascontiguousarray(
        np.transpose(np.asarray(lambda_B, np.float32), (0, 3, 1, 2)).reshape(
            L - 1, G * C, M))
    return {"lamB0p": lamB0p, "lamPip": lamPip, "lamQp": lamQp, "lamBp": lamBp}


def make_consts():
    iota_f = np.tile(np.arange(128, dtype=np.float32), (128, 1))
    iota_b = iota_f.astype(BF)
    ident_f = np.eye(128, dtype=np.float32)
    # maskg[p, f] = 1 if p//8 == f//8 (same-g block for Qbig expansion)
    pp = np.arange(128) // 8
    maskg = (pp[:, None] == pp[None, :]).astype(np.float32)
    return {"iota_f": iota_f, "iota_b": iota_b, "ident_f": ident_f,
            "maskg": maskg}


# ---- bass kernel builder ----------------------------------------------------

def build_nc(cfg, TA, TB):
    import concourse.bass as bass
    import concourse.bacc as bacc
    import concourse.mybir as mybir
    import concourse.tile as tile

    fp32 = mybir.dt.float32
    bf16 = mybir.dt.bfloat16
    i16 = mybir.dt.int16
    AX = mybir.AxisListType.X
    OP = mybir.AluOpType
    AF = mybir.ActivationFunctionType

    nb, npc, half, tg = cfg.nb, cfg.npc, cfg.half, cfg.tg
    totTA, totTB = int(np.sum(TA)), int(np.sum(TB))
    T_tot = totTA + totTB
    cumA = np.concatenate([[0], np.cumsum(TA)]).astype(int)
    cumB = np.concatenate([[0], np.cumsum(TB)]).astype(int)
    last_nn = npc - (nb - 1) * 128

    nc = bacc.Bacc("TRN2", target_bir_lowering=False, debug=False,
                   num_devices=cfg.ncores)

    # ---- dram I/O
    idx_d = nc.dram_tensor("idx", [128, T_tot * 8], i16, kind="ExternalInput")
    dstloc_d = nc.dram_tensor("dstloc", [128, T_tot], fp32, kind="ExternalInput")
    x_d = nc.dram_tensor("xq", [128, nb], fp32, kind="ExternalInput")
    lam_B0 = nc.dram_tensor("lamB0p", [128, M], fp32, kind="ExternalInput")
    lam_Pi = nc.dram_tensor("lamPip", [G, C], fp32, kind="ExternalInput")
    lam_Q = nc.dram_tensor("lamQp", [L - 1, 128, C], fp32, kind="ExternalInput")
    lam_B = nc.dram_tensor("lamBp", [L - 1, 128, M], fp32, kind="ExternalInput")
    pi_bounce = nc.dram_tensor("pi_bounce", [G * C], fp32)
    iota_f_d = nc.dram_tensor("iota_f", [128, 128], fp32, kind="ExternalInput")
    iota_b_d = nc.dram_tensor("iota_b", [128, 128], bf16, kind="ExternalInput")
    ident_f_d = nc.dram_tensor("ident_f", [128, 128], fp32, kind="ExternalInput")
    maskg_d = nc.dram_tensor("maskg", [128, 128], fp32, kind="ExternalInput")
    lls_d = nc.dram_tensor("lls", [npc, L * G], fp32, kind="ExternalOutput")

    h_slice = [nc.dram_tensor(f"h_slice{l}", [npc, CG], bf16) for l in range(L - 1)]
    h_full = [nc.dram_tensor(f"h_full{l}", [cfg.n, CG], bf16, addr_space="Shared")
              for l in range(L - 1)]
    rgroups = [list(range(cfg.ncores))]

    with tile.TileContext(nc) as tc:
        from contextlib import ExitStack
        with ExitStack() as ctx:
            res = ctx.enter_context(tc.tile_pool(name="res", bufs=1))
            sbp = ctx.enter_context(tc.tile_pool(name="sbp", bufs=3))
            ohp = ctx.enter_context(tc.tile_pool(name="ohp", bufs=12))
            gpA = ctx.enter_context(tc.tile_pool(name="gpA", bufs=2))
            gpB = ctx.enter_context(tc.tile_pool(name="gpB", bufs=2))
            psp = ctx.enter_context(tc.tile_pool(name="psp", bufs=2, space="PSUM"))

            # ---- residents
            iota_f = res.tile([128, 128], fp32)
            nc.sync.dma_start(out=iota_f[:], in_=iota_f_d[:])
            iota_b = res.tile([128, 128], bf16)
            nc.sync.dma_start(out=iota_b[:], in_=iota_b_d[:])
            ident_f = res.tile([128, 128], fp32)
            nc.sync.dma_start(out=ident_f[:], in_=ident_f_d[:])
            maskg = res.tile([128, 128], fp32)
            nc.sync.dma_start(out=maskg[:], in_=maskg_d[:])
            idx_t = res.tile([128, T_tot * 8], i16)
            nc.sync.dma_start(out=idx_t[:], in_=idx_d[:])
            dstloc = res.tile([128, T_tot], fp32)
            nc.sync.dma_start(out=dstloc[:], in_=dstloc_d[:])
            x_t = res.tile([128, nb], fp32)
            nc.sync.dma_start(out=x_t[:], in_=x_d[:])
            ohXT = res.tile([32, nb * 128], fp32)     # one-hot(x)^T, all blocks
            out_sb = res.tile([128, nb * 64], fp32)   # lls accumulator
            qbig = res.tile([128, 128], fp32)
            barrT = res.tile([32, 128], fp32)         # layer's B table [m, cg]
            pi_col = res.tile([128, 1], fp32)

            def softmax_free(raw, nfree, tag):
                """softmax over free dim of raw [128p, nfree] fp32 -> new tile"""
                mx = sbp.tile([raw.shape[0], 1], fp32, tag=f"{tag}mx")
                nc.vector.tensor_reduce(out=mx[:], in_=raw[:], axis=AX,
                                        op=OP.max, negate=True)
                ex = sbp.tile([raw.shape[0], nfree], fp32, tag=f"{tag}ex")
                nc.scalar.activation(out=ex[:], in_=raw[:], func=AF.Exp,
                                     bias=mx[:, 0:1], scale=1.0)
                sm = sbp.tile([raw.shape[0], 1], fp32, tag=f"{tag}sm")
                nc.vector.reduce_sum(out=sm[:], in_=ex[:], axis=AX)
                rs = sbp.tile([raw.shape[0], 1], fp32, tag=f"{tag}rs")
                nc.vector.reciprocal(out=rs[:], in_=sm[:])
                out = sbp.tile([raw.shape[0], nfree], fp32, tag=f"{tag}out")
                nc.vector.tensor_scalar(out=out[:], in0=ex[:], scalar1=rs[:, 0:1],
                                        scalar2=None, op0=OP.mult)
                return out

            def prep_BarrT(src_ap, dest):
                """lambda_B-like [C, M, G] -> dest [32, 128] fp32 = B^T[m, (g c)],
                softmax over M; optionally scaled by pi_col."""
                raw = sbp.tile([128, M], fp32, tag="braw")
                nc.sync.dma_start(out=raw[:], in_=src_ap)
                bsm = softmax_free(raw, M, "b")
                return bsm

            def transpose_to(dest_sb, src_sb, pdim, fdim):
                """dest_sb [fdim, pdim] <- src_sb [pdim, fdim]^T via PE"""
                ps = psp.tile([fdim, pdim], fp32, tag="trp", space="PSUM")
                nc.tensor.transpose(out=ps[:], in_=src_sb[:],
                                    identity=ident_f[:pdim, :pdim])
                nc.scalar.copy(out=dest_sb[:], in_=ps[:])

            # ================= layer 0 =================
            # B0P[cg, m] = softmax_M(lambda_B0)[c,m,g] * Pi[c,g];  [(g c), m]
            b0sm = prep_BarrT(lam_B0[:], None)
            # Pi: [16, 8] softmax over free c, then scatter to [128, 1]
            praw = sbp.tile([16, C], fp32, tag="praw")
            nc.sync.dma_start(out=praw[:], in_=lam_Pi[:])
            pism = softmax_free(praw, C, "p")
            nc.sync.dma_start(out=pi_bounce[:].rearrange("(g c) -> g c", c=C),
                              in_=pism[:])
            nc.sync.dma_start(out=pi_col[:], in_=pi_bounce[:, None])
            b0p = sbp.tile([128, M], fp32, tag="b0p")
            nc.vector.tensor_scalar(out=b0p[:], in0=b0sm[:], scalar1=pi_col[:, 0:1],
                                    scalar2=None, op0=OP.mult)
            transpose_to(barrT, b0p, 128, 32)  # barrT <- B0P^T [m=32, cg]

            for b in range(nb):
                nn = 128 if b < nb - 1 else last_nn
                oh32 = sbp.tile([128, 32], fp32, tag="oh32")
                nc.vector.tensor_scalar(out=oh32[:], in0=iota_f[:, :32],
                                        scalar1=x_t[:, b:b + 1], scalar2=None,
                                        op0=OP.is_equal)
                trp = psp.tile([32, 128], fp32, tag="trp", space="PSUM")
                nc.tensor.transpose(out=trp[:], in_=oh32[:], identity=ident_f[:])
                nc.scalar.copy(out=ohXT[:, b * 128:(b + 1) * 128], in_=trp[:])
                u0p = psp.tile([128, 128], fp32, tag="bx", space="PSUM")
                nc.tensor.matmul(out=u0p[:], lhsT=ohXT[:, b * 128:(b + 1) * 128],
                                 rhs=barrT[:], start=True, stop=True)
                u = sbp.tile([128, 128], fp32, tag="u")
                nc.scalar.copy(out=u[:], in_=u0p[:])
                Z = sbp.tile([128, G], fp32, tag="Z")
                nc.vector.reduce_sum(out=Z[:], in_=u[:].rearrange(
                    "p (g c) -> p g c", c=C), axis=AX)
                nc.scalar.activation(out=out_sb[:, b * 64:b * 64 + G], in_=Z[:],
                                     func=AF.Ln)
                rz = sbp.tile([128, G], fp32, tag="rz")
                nc.vector.reciprocal(out=rz[:], in_=Z[:])
                h = sbp.tile([128, 128], bf16, tag="h")
                nc.vector.tensor_tensor(
                    out=h[:].rearrange("p (g c) -> p g c", c=C),
                    in0=u[:].rearrange("p (g c) -> p g c", c=C),
                    in1=rz[:].to_broadcast([128, G, C]), op=OP.mult)
                nc.sync.dma_start(out=h_slice[0][b * 128:b * 128 + nn, :],
                                  in_=h[:nn, :])

            # ================= graph layers =================
            for l in range(1, L):
                lq = l - 1
                # all-gather previous h
                nc.gpsimd.collective_compute(
                    "AllGather", OP.bypass, replica_groups=rgroups,
                    ins=[h_slice[lq][:]], outs=[h_full[lq][:]])

                # ---- layer params
                qraw = sbp.tile([128, C], fp32, tag="qraw")
                nc.sync.dma_start(out=qraw[:], in_=lam_Q[lq])
                qsm = softmax_free(qraw, C, "q")  # [(g k), c]
                qsm_ap = qsm[:]
                qsm_bc = bass.AP(qsm_ap.tensor, qsm_ap.offset,
                                 [qsm_ap.ap[0], [0, G], qsm_ap.ap[1]])
                nc.vector.tensor_tensor(
                    out=qbig[:].rearrange("p (g c) -> p g c", c=C),
                    in0=qsm_bc,
                    in1=maskg[:].rearrange("p (g c) -> p g c", c=C),
                    op=OP.mult)
                bsm = prep_BarrT(lam_B[lq], None)
                transpose_to(barrT, bsm, 128, 32)

                # ---- gather chunk management
                nchA = -(-totTA // tg)
                chunk_cache = [{}, {}]

                def get_tile(stream, t_idx, l=l, lq=lq):
                    pool = gpA if stream == 0 else gpB
                    tot = totTA if stream == 0 else totTB
                    tab = h_full[lq][:half, :] if stream == 0 else h_full[lq][half:, :]
                    colb = 0 if stream == 0 else totTA * 8
                    cache = chunk_cache[stream]
                    ci = t_idx // tg
                    if ci not in cache:
                        ntile = min(tg, tot - ci * tg)
                        buf = pool.tile([128, ntile * 128], bf16,
                                        tag=f"g{stream}")
                        nc.gpsimd.dma_gather(
                            out_ap=buf[:].rearrange("p (t e) -> p t e", e=128),
                            in_ap=tab,
                            idxs_ap=idx_t[:, colb + ci * tg * 8:
                                          colb + (ci * tg + ntile) * 8],
                            num_idxs=ntile * 128,
                            num_idxs_reg=ntile * 128,
                            elem_size=128,
                            single_packet=False)
                        cache[ci] = buf
                    return cache[ci][:].rearrange("p (t e) -> p t e", e=128)[
                        :, t_idx - ci * tg, :]

                for b in range(nb):
                    nn = 128 if b < nb - 1 else last_nn
                    agg = psp.tile([128, 128], fp32, tag="agg", space="PSUM")
                    nt = int(TA[b] + TB[b])
                    i = 0
                    for stream, cum in ((0, cumA), (1, cumB)):
                        Tb = int(TA[b] if stream == 0 else TB[b])
                        colb = 0 if stream == 0 else totTA
                        for t in range(Tb):
                            gt = int(cum[b]) + t
                            gat = get_tile(stream, gt)
                            oh = ohp.tile([128, 128], bf16, tag="oh")
                            nc.vector.tensor_scalar(
                                out=oh[:], in0=iota_b[:],
                                scalar1=dstloc[:, colb + gt:colb + gt + 1],
                                scalar2=None, op0=OP.is_equal)
                            nc.tensor.matmul(out=agg[:], lhsT=oh[:], rhs=gat,
                                             start=(i == 0), stop=(i == nt - 1))
                            i += 1

                    aggsb = sbp.tile([128, 128], fp32, tag="aggsb")
                    nc.scalar.copy(out=aggsb[:], in_=agg[:])
                    cnt = sbp.tile([128, 1], fp32, tag="cnt")
                    nc.vector.reduce_sum(out=cnt[:], in_=aggsb[:], axis=AX)
                    logcnt = sbp.tile([128, 1], fp32, tag="logcnt")
                    nc.scalar.activation(out=logcnt[:], in_=cnt[:], func=AF.Ln,
                                         scale=1.0 / G)
                    # QA^T = Qbig^T(lhsT=qbig) @ aggr^T
                    trp = psp.tile([128, 128], fp32, tag="trp", space="PSUM")
                    nc.tensor.transpose(out=trp[:], in_=aggsb[:],
                                        identity=ident_f[:])
                    aggT = sbp.tile([128, 128], fp32, tag="aggT")
                    nc.scalar.copy(out=aggT[:], in_=trp[:])
                    qaT = psp.tile([128, 128], fp32, tag="qa", space="PSUM")
                    nc.tensor.matmul(out=qaT[:], lhsT=qbig[:], rhs=aggT[:],
                                     start=True, stop=True)
                    qaTsb = sbp.tile([128, 128], fp32, tag="qaTsb")
                    nc.scalar.copy(out=qaTsb[:], in_=qaT[:])
                    qa2 = psp.tile([128, 128], fp32, tag="trp", space="PSUM")
                    nc.tensor.transpose(out=qa2[:], in_=qaTsb[:],
                                        identity=ident_f[:])
                    bx = psp.tile([128, 128], fp32, tag="bx", space="PSUM")
                    nc.tensor.matmul(out=bx[:],
                                     lhsT=ohXT[:, b * 128:(b + 1) * 128],
                                     rhs=barrT[:], start=True, stop=True)
                    bxsb = sbp.tile([128, 128], fp32, tag="bxsb")
                    nc.scalar.copy(out=bxsb[:], in_=bx[:])
                    u = sbp.tile([128, 128], fp32, tag="u")
                    nc.vector.tensor_tensor(out=u[:], in0=qa2[:], in1=bxsb[:],
                                            op=OP.mult)
                    Z = sbp.tile([128, G], fp32, tag="Z")
                    nc.vector.reduce_sum(out=Z[:], in_=u[:].rearrange(
                        "p (g c) -> p g c", c=C), axis=AX)
                    logZ = sbp.tile([128, G], fp32, tag="logZ")
                    nc.scalar.activation(out=logZ[:], in_=Z[:], func=AF.Ln)
                    nc.vector.tensor_scalar(
                        out=out_sb[:, b * 64 + l * G:b * 64 + (l + 1) * G],
                        in0=logZ[:], scalar1=logcnt[:, 0:1], scalar2=None,
                        op0=OP.subtract)
                    if l < L - 1:
                        rz = sbp.tile([128, G], fp32, tag="rz")
                        nc.vector.reciprocal(out=rz[:], in_=Z[:])
                        h = sbp.tile([128, 128], bf16, tag="h")
                        nc.vector.tensor_tensor(
                            out=h[:].rearrange("p (g c) -> p g c", c=C),
                            in0=u[:].rearrange("p (g c) -> p g c", c=C),
                            in1=rz[:].to_broadcast([128, G, C]), op=OP.mult)
                        nc.sync.dma_start(
                            out=h_slice[l][b * 128:b * 128 + nn, :],
                            in_=h[:nn, :])

            # ---- write lls out
            if nb > 1:
                nc.sync.dma_start(
                    out=lls_d[:(nb - 1) * 128, :].rearrange(
                        "(b p) c -> p b c", p=128),
                    in_=out_sb[:].rearrange("p (b c) -> p b c", c=64)[:, :nb - 1, :])
            nc.sync.dma_start(
                out=lls_d[(nb - 1) * 128:, :],
                in_=out_sb[:last_nn, (nb - 1) * 64:nb * 64])

    nc.compile()
    return nc


# ---- entry point ------------------------------------------------------------

def kernel(x, edge_index, lambda_B0, lambda_Pi, lambda_Q, lambda_B):
    cfg = Cfg()
    cores, TA, TB = preprocess(x, edge_index, cfg)
    consts = make_consts()
    nc = build_nc(cfg, TA, TB)

    from concourse.bass_utils import run_bass_kernel_spmd
    params = permute_params(lambda_B0, lambda_Pi, lambda_Q, lambda_B)
    in_maps = []
    for c in range(cfg.ncores):
        m = dict(cores[c])
        m.update(params)
        m.update({k: np.ascontiguousarray(v) for k, v in consts.items()})
        in_maps.append(m)

    res = run_bass_kernel_spmd(nc, in_maps, core_ids=list(range(cfg.ncores)))
    out = np.concatenate([res.results[c]["lls"] for c in range(cfg.ncores)],
                         axis=0)
    return out.reshape(N, L, G).astype(np.float32)


# revision 9
# speedup vs baseline: 1.2641x; 1.2641x over previous
"""CGMM (Contextual Graph Markov Model) forward pass on 8 Trainium2 NeuronCores.

Self-contained: takes FULL inputs as numpy arrays, shards nodes/edges across
the 8 cores (graph parallel), runs a Bass/Tile kernel via
run_bass_kernel_spmd, returns the FULL [N, L, G] log-likelihood output.

Algorithm layout (per core, nodes on partitions, cg = g*8 + c on free dim):
  layer 0:  u0[n, cg] = B0[c, x_n, g]*Pi[c, g]  via one-hot(x) matmul
            Z = sum_c u, ll0 = log Z, h = u/Z  (h stored bf16, row-major)
  layers 1..3:
            all-gather h across cores  ->  h_full [N, 128] bf16 (Shared DRAM)
            gather h_full[src] per edge (dma_gather, 256B rows)
            aggr[dst, cg] = segment-sum via one-hot(dst_local) matmuls (PSUM fp32)
            cnt from row-sums of aggr (h rows sum to G exactly)
            QA = Qbig @ aggr^T (PE transpose + fp32 matmul)
            u = Bx * QA; Z = sum_c u; ll = log Z - log(cnt); h = u/Z
Edge streams are host-preprocessed: sorted by (dst block, src half), padded to
a cross-core-uniform tile schedule; padded slots gather row 0 with
dst_local = -1 (one-hot row of zeros -> no contribution).
"""
import os
import sys

sys.path.insert(0, "/opt/trn_rl_repo")

import numpy as np
import ml_dtypes

BF = ml_dtypes.bfloat16

# ---- problem sizes (hardcoded per contract) --------------------------------
N, E, C, M, G, L = 50000, 800000, 8, 32, 16, 4
NCORES = 8
CG = C * G  # 128


class Cfg:
    def __init__(self, n=N, e=E, ncores=NCORES, tg=32):
        self.n = n
        self.e = e
        self.ncores = ncores
        self.npc = n // ncores
        self.nb = (self.npc + 127) // 128
        self.half = n // 2
        self.tg = tg  # gather chunk size in 128-edge tiles
        self.lo_nb = (self.nb + 1) // 2  # blocks in the lo bank


# ---- host preprocessing -----------------------------------------------------

def preprocess(x, edge_index, cfg):
    """Build per-core aux arrays + the (cross-core uniform) tile schedule."""
    dst = np.asarray(edge_index[0], dtype=np.int64)
    src = np.asarray(edge_index[1], dtype=np.int64)
    x = np.asarray(x, dtype=np.int64)
    nc_, npc, nb, half = cfg.ncores, cfg.npc, cfg.nb, cfg.half

    lo_nb = cfg.lo_nb
    LO = lo_nb * 128
    HI = npc - LO
    owner = dst // npc
    per_core = []
    cntAB = np.zeros((nc_, nb, 2), dtype=np.int64)
    for c in range(nc_):
        sel = owner == c
        d = dst[sel] - c * npc
        s = src[sel]
        b = d // 128
        order = np.argsort(b, kind="stable")
        b, d, s = b[order], d[order], s[order]
        dl = d % 128
        sown = s // npc
        soff = s % npc
        hf = (soff >= LO).astype(np.int64)
        # bank row ids
        s = np.where(hf == 0, sown * LO + soff, sown * HI + (soff - LO))
        per_core.append((b, dl, s, hf))
        # counts per (block, half)
        key = b * 2 + hf
        cnt = np.bincount(key, minlength=nb * 2).reshape(nb, 2)
        cntAB[c] = cnt
    TA = np.maximum(1, -(-cntAB[:, :, 0].max(axis=0) // 128))
    TB = np.maximum(1, -(-cntAB[:, :, 1].max(axis=0) // 128))
    totTA, totTB = int(TA.sum()), int(TB.sum())
    offA = np.concatenate([[0], np.cumsum(TA)]).astype(np.int64)  # tile offsets
    offB = np.concatenate([[0], np.cumsum(TB)]).astype(np.int64)

    cores = []
    for c in range(nc_):
        b, dl, s, hf = per_core[c]
        idxA = np.zeros(totTA * 128, dtype=np.int64)
        dlA = np.full(totTA * 128, -1, dtype=np.int64)
        idxB = np.zeros(totTB * 128, dtype=np.int64)
        dlB = np.full(totTB * 128, -1, dtype=np.int64)
        for bb in range(nb):
            mA = (b == bb) & (hf == 0)
            mB = (b == bb) & (hf == 1)
            nA, nB_ = int(mA.sum()), int(mB.sum())
            a0, b0 = offA[bb] * 128, offB[bb] * 128
            idxA[a0:a0 + nA] = s[mA]
            dlA[a0:a0 + nA] = dl[mA]
            idxB[b0:b0 + nB_] = s[mB]
            dlB[b0:b0 + nB_] = dl[mB]

        # idx dram layout: [128, cols] int16; index i at [i%16, i//16], the
        # 16-row block replicated 8x down the partitions (one copy per Q7 core)
        allidx = np.concatenate([idxA, idxB]).astype(np.int16)
        idx16 = allidx.reshape(-1, 16).T  # [16, tot/16]
        idx_d = np.tile(idx16, (8, 1))    # [128, tot/16]

        # dstloc dram layout: [128, T_tot] bf16, partition = slot within tile
        alldl = np.concatenate([dlA, dlB]).astype(np.float32)
        dl_d = alldl.reshape(-1, 128).T.copy()  # [128, T_tot] fp32

        # x dram layout: [128, nb], partition-major, fp32
        xloc = np.zeros(nb * 128, dtype=np.float32)
        xloc[:npc] = x[c * npc:(c + 1) * npc]
        x_d = xloc.reshape(nb, 128).T.copy()  # [128, nb]

        cores.append({"idx": np.ascontiguousarray(idx_d),
                      "dstloc": np.ascontiguousarray(dl_d),
                      "xq": np.ascontiguousarray(x_d)})
    return cores, TA.astype(int), TB.astype(int)


def permute_params(lambda_B0, lambda_Pi, lambda_Q, lambda_B):
    """Pure layout permutations (no compute): partition (g, c/k)-major views."""
    lamB0p = np.ascontiguousarray(
        np.transpose(np.asarray(lambda_B0, np.float32), (2, 0, 1)).reshape(G * C, M))
    lamPip = np.ascontiguousarray(np.asarray(lambda_Pi, np.float32).T)  # [G, C]
    lamQp = np.ascontiguousarray(
        np.transpose(np.asarray(lambda_Q, np.float32), (0, 3, 2, 1)).reshape(
            L - 1, G * C, C))
    lamBp = np.ascontiguousarray(
        np.transpose(np.asarray(lambda_B, np.float32), (0, 3, 1, 2)).reshape(
            L - 1, G * C, M))
    return {"lamB0p": lamB0p, "lamPip": lamPip, "lamQp": lamQp, "lamBp": lamBp}


def make_consts():
    iota_f = np.tile(np.arange(128, dtype=np.float32), (128, 1))
    iota_b = iota_f.astype(BF)
    ident_f = np.eye(128, dtype=np.float32)
    # maskg[p, f] = 1 if p//8 == f//8 (same-g block for Qbig expansion)
    pp = np.arange(128) // 8
    maskg = (pp[:, None] == pp[None, :]).astype(np.float32)
    return {"iota_f": iota_f, "iota_b": iota_b, "ident_f": ident_f,
            "maskg": maskg}


# ---- bass kernel builder ----------------------------------------------------

def build_nc(cfg, TA, TB):
    import concourse.bass as bass
    import concourse.bacc as bacc
    import concourse.mybir as mybir
    import concourse.tile as tile

    fp32 = mybir.dt.float32
    bf16 = mybir.dt.bfloat16
    i16 = mybir.dt.int16
    AX = mybir.AxisListType.X
    OP = mybir.AluOpType
    AF = mybir.ActivationFunctionType

    nb, npc, half, tg = cfg.nb, cfg.npc, cfg.half, cfg.tg
    totTA, totTB = int(np.sum(TA)), int(np.sum(TB))
    T_tot = totTA + totTB
    cumA = np.concatenate([[0], np.cumsum(TA)]).astype(int)
    cumB = np.concatenate([[0], np.cumsum(TB)]).astype(int)
    last_nn = npc - (nb - 1) * 128

    nc = bacc.Bacc("TRN2", target_bir_lowering=False, debug=False,
                   num_devices=cfg.ncores)

    # ---- dram I/O
    idx_d = nc.dram_tensor("idx", [128, T_tot * 8], i16, kind="ExternalInput")
    dstloc_d = nc.dram_tensor("dstloc", [128, T_tot], fp32, kind="ExternalInput")
    x_d = nc.dram_tensor("xq", [128, nb], fp32, kind="ExternalInput")
    lam_B0 = nc.dram_tensor("lamB0p", [128, M], fp32, kind="ExternalInput")
    lam_Pi = nc.dram_tensor("lamPip", [G, C], fp32, kind="ExternalInput")
    lam_Q = nc.dram_tensor("lamQp", [L - 1, 128, C], fp32, kind="ExternalInput")
    lam_B = nc.dram_tensor("lamBp", [L - 1, 128, M], fp32, kind="ExternalInput")
    pi_bounce = nc.dram_tensor("pi_bounce", [G * C], fp32)
    iota_f_d = nc.dram_tensor("iota_f", [128, 128], fp32, kind="ExternalInput")
    iota_b_d = nc.dram_tensor("iota_b", [128, 128], bf16, kind="ExternalInput")
    ident_f_d = nc.dram_tensor("ident_f", [128, 128], fp32, kind="ExternalInput")
    maskg_d = nc.dram_tensor("maskg", [128, 128], fp32, kind="ExternalInput")
    lls_d = nc.dram_tensor("lls", [npc, L * G], fp32, kind="ExternalOutput")

    lo_nb = cfg.lo_nb
    LO = lo_nb * 128
    HI = npc - LO
    h_slice_lo = [nc.dram_tensor(f"h_slo{l}", [LO, CG], bf16) for l in range(L - 1)]
    h_slice_hi = [nc.dram_tensor(f"h_shi{l}", [HI, CG], bf16) for l in range(L - 1)]
    h_full_lo = [nc.dram_tensor(f"h_flo{l}", [cfg.ncores * LO, CG], bf16,
                                addr_space="Shared") for l in range(L - 1)]
    h_full_hi = [nc.dram_tensor(f"h_fhi{l}", [cfg.ncores * HI, CG], bf16,
                                addr_space="Shared") for l in range(L - 1)]
    rgroups = [list(range(cfg.ncores))]

    with tile.TileContext(nc) as tc:
        from contextlib import ExitStack
        with ExitStack() as ctx:
            res = ctx.enter_context(tc.tile_pool(name="res", bufs=1))
            sbp = ctx.enter_context(tc.tile_pool(name="sbp", bufs=3))
            ohp = ctx.enter_context(tc.tile_pool(name="ohp", bufs=12))
            gpA = ctx.enter_context(tc.tile_pool(name="gpA", bufs=3))
            gpB = ctx.enter_context(tc.tile_pool(name="gpB", bufs=3))
            psp = ctx.enter_context(tc.tile_pool(name="psp", bufs=2, space="PSUM"))

            # ---- residents
            iota_f = res.tile([128, 128], fp32)
            nc.sync.dma_start(out=iota_f[:], in_=iota_f_d[:])
            iota_b = res.tile([128, 128], bf16)
            nc.sync.dma_start(out=iota_b[:], in_=iota_b_d[:])
            ident_f = res.tile([128, 128], fp32)
            nc.sync.dma_start(out=ident_f[:], in_=ident_f_d[:])
            maskg = res.tile([128, 128], fp32)
            nc.sync.dma_start(out=maskg[:], in_=maskg_d[:])
            idx_t = res.tile([128, T_tot * 8], i16)
            nc.sync.dma_start(out=idx_t[:], in_=idx_d[:])
            dstloc = res.tile([128, T_tot], fp32)
            nc.sync.dma_start(out=dstloc[:], in_=dstloc_d[:])
            x_t = res.tile([128, nb], fp32)
            nc.sync.dma_start(out=x_t[:], in_=x_d[:])
            ohXT = res.tile([32, nb * 128], fp32)     # one-hot(x)^T, all blocks
            out_sb = res.tile([128, nb * 64], fp32)   # lls accumulator
            qbig = res.tile([128, 128], fp32)
            barrT = res.tile([32, 128], fp32)         # layer's B table [m, cg]
            pi_col = res.tile([128, 1], fp32)

            def softmax_free(raw, nfree, tag):
                """softmax over free dim of raw [128p, nfree] fp32 -> new tile"""
                mx = sbp.tile([raw.shape[0], 1], fp32, tag=f"{tag}mx")
                nc.vector.tensor_reduce(out=mx[:], in_=raw[:], axis=AX,
                                        op=OP.max, negate=True)
                ex = sbp.tile([raw.shape[0], nfree], fp32, tag=f"{tag}ex")
                nc.scalar.activation(out=ex[:], in_=raw[:], func=AF.Exp,
                                     bias=mx[:, 0:1], scale=1.0)
                sm = sbp.tile([raw.shape[0], 1], fp32, tag=f"{tag}sm")
                nc.vector.reduce_sum(out=sm[:], in_=ex[:], axis=AX)
                rs = sbp.tile([raw.shape[0], 1], fp32, tag=f"{tag}rs")
                nc.vector.reciprocal(out=rs[:], in_=sm[:])
                out = sbp.tile([raw.shape[0], nfree], fp32, tag=f"{tag}out")
                nc.vector.tensor_scalar(out=out[:], in0=ex[:], scalar1=rs[:, 0:1],
                                        scalar2=None, op0=OP.mult)
                return out

            def prep_BarrT(src_ap, dest):
                """lambda_B-like [C, M, G] -> dest [32, 128] fp32 = B^T[m, (g c)],
                softmax over M; optionally scaled by pi_col."""
                raw = sbp.tile([128, M], fp32, tag="braw")
                nc.sync.dma_start(out=raw[:], in_=src_ap)
                bsm = softmax_free(raw, M, "b")
                return bsm

            def transpose_to(dest_sb, src_sb, pdim, fdim):
                """dest_sb [fdim, pdim] <- src_sb [pdim, fdim]^T via PE"""
                ps = psp.tile([fdim, pdim], fp32, tag="trp", space="PSUM")
                nc.tensor.transpose(out=ps[:], in_=src_sb[:],
                                    identity=ident_f[:pdim, :pdim])
                nc.scalar.copy(out=dest_sb[:], in_=ps[:])

            # ================= layer 0 =================
            # B0P[cg, m] = softmax_M(lambda_B0)[c,m,g] * Pi[c,g];  [(g c), m]
            b0sm = prep_BarrT(lam_B0[:], None)
            # Pi: [16, 8] softmax over free c, then scatter to [128, 1]
            praw = sbp.tile([16, C], fp32, tag="praw")
            nc.sync.dma_start(out=praw[:], in_=lam_Pi[:])
            pism = softmax_free(praw, C, "p")
            nc.sync.dma_start(out=pi_bounce[:].rearrange("(g c) -> g c", c=C),
                              in_=pism[:])
            nc.sync.dma_start(out=pi_col[:], in_=pi_bounce[:, None])
            b0p = sbp.tile([128, M], fp32, tag="b0p")
            nc.vector.tensor_scalar(out=b0p[:], in0=b0sm[:], scalar1=pi_col[:, 0:1],
                                    scalar2=None, op0=OP.mult)
            transpose_to(barrT, b0p, 128, 32)  # barrT <- B0P^T [m=32, cg]

            for b in range(nb):
                nn = 128 if b < nb - 1 else last_nn
                oh32 = sbp.tile([128, 32], fp32, tag="oh32")
                nc.vector.tensor_scalar(out=oh32[:], in0=iota_f[:, :32],
                                        scalar1=x_t[:, b:b + 1], scalar2=None,
                                        op0=OP.is_equal)
                trp = psp.tile([32, 128], fp32, tag="trp", space="PSUM")
                nc.tensor.transpose(out=trp[:], in_=oh32[:], identity=ident_f[:])
                nc.scalar.copy(out=ohXT[:, b * 128:(b + 1) * 128], in_=trp[:])
                u0p = psp.tile([128, 128], fp32, tag="bx", space="PSUM")
                nc.tensor.matmul(out=u0p[:], lhsT=ohXT[:, b * 128:(b + 1) * 128],
                                 rhs=barrT[:], start=True, stop=True)
                u = sbp.tile([128, 128], fp32, tag="u")
                nc.scalar.copy(out=u[:], in_=u0p[:])
                Z = sbp.tile([128, G], fp32, tag="Z")
                nc.vector.reduce_sum(out=Z[:], in_=u[:].rearrange(
                    "p (g c) -> p g c", c=C), axis=AX)
                nc.scalar.activation(out=out_sb[:, b * 64:b * 64 + G], in_=Z[:],
                                     func=AF.Ln)
                rz = sbp.tile([128, G], fp32, tag="rz")
                nc.vector.reciprocal(out=rz[:], in_=Z[:])
                h = sbp.tile([128, 128], bf16, tag="h")
                nc.vector.tensor_tensor(
                    out=h[:].rearrange("p (g c) -> p g c", c=C),
                    in0=u[:].rearrange("p (g c) -> p g c", c=C),
                    in1=rz[:].to_broadcast([128, G, C]), op=OP.mult)
                if b < lo_nb:
                    nc.sync.dma_start(out=h_slice_lo[0][b * 128:b * 128 + nn, :],
                                      in_=h[:nn, :])
                else:
                    bo = b - lo_nb
                    nc.sync.dma_start(out=h_slice_hi[0][bo * 128:bo * 128 + nn, :],
                                      in_=h[:nn, :])
                if b == lo_nb - 1:
                    nc.gpsimd.collective_compute(
                        "AllGather", OP.bypass, replica_groups=rgroups,
                        ins=[h_slice_lo[0][:]], outs=[h_full_lo[0][:]])

            # ================= graph layers =================
            for l in range(1, L):
                lq = l - 1
                # all-gather previous h's hi bank (lo AG was traced mid-prev-layer)
                nc.gpsimd.collective_compute(
                    "AllGather", OP.bypass, replica_groups=rgroups,
                    ins=[h_slice_hi[lq][:]], outs=[h_full_hi[lq][:]])

                # ---- layer params
                qraw = sbp.tile([128, C], fp32, tag="qraw")
                nc.sync.dma_start(out=qraw[:], in_=lam_Q[lq])
                qsm = softmax_free(qraw, C, "q")  # [(g k), c]
                qsm_ap = qsm[:]
                qsm_bc = bass.AP(qsm_ap.tensor, qsm_ap.offset,
                                 [qsm_ap.ap[0], [0, G], qsm_ap.ap[1]])
                nc.vector.tensor_tensor(
                    out=qbig[:].rearrange("p (g c) -> p g c", c=C),
                    in0=qsm_bc,
                    in1=maskg[:].rearrange("p (g c) -> p g c", c=C),
                    op=OP.mult)
                bsm = prep_BarrT(lam_B[lq], None)
                transpose_to(barrT, bsm, 128, 32)

                # ---- gather chunk management
                nchA = -(-totTA // tg)
                chunk_cache = [{}, {}]

                def get_tile(stream, t_idx, l=l, lq=lq):
                    pool = gpA if stream == 0 else gpB
                    tot = totTA if stream == 0 else totTB
                    tab = h_full_lo[lq][:] if stream == 0 else h_full_hi[lq][:]
                    colb = 0 if stream == 0 else totTA * 8
                    cache = chunk_cache[stream]
                    ci = t_idx // tg
                    if ci not in cache:
                        ntile = min(tg, tot - ci * tg)
                        buf = pool.tile([128, ntile * 128], bf16,
                                        tag=f"g{stream}")
                        nc.gpsimd.dma_gather(
                            out_ap=buf[:].rearrange("p (t e) -> p t e", e=128),
                            in_ap=tab,
                            idxs_ap=idx_t[:, colb + ci * tg * 8:
                                          colb + (ci * tg + ntile) * 8],
                            num_idxs=ntile * 128,
                            num_idxs_reg=ntile * 128,
                            elem_size=128,
                            single_packet=False)
                        cache[ci] = buf
                    return cache[ci][:].rearrange("p (t e) -> p t e", e=128)[
                        :, t_idx - ci * tg, :]

                for ci in range(min(3, -(-totTA // tg))):
                    get_tile(0, ci * tg)

                for b in range(nb):
                    nn = 128 if b < nb - 1 else last_nn
                    agg = psp.tile([128, 128], fp32, tag="agg", space="PSUM")
                    nt = int(TA[b] + TB[b])
                    i = 0
                    for stream, cum in ((0, cumA), (1, cumB)):
                        Tb = int(TA[b] if stream == 0 else TB[b])
                        colb = 0 if stream == 0 else totTA
                        for t in range(Tb):
                            gt = int(cum[b]) + t
                            gat = get_tile(stream, gt)
                            oh = ohp.tile([128, 128], bf16, tag="oh")
                            nc.vector.tensor_scalar(
                                out=oh[:], in0=iota_b[:],
                                scalar1=dstloc[:, colb + gt:colb + gt + 1],
                                scalar2=None, op0=OP.is_equal)
                            nc.tensor.matmul(out=agg[:], lhsT=oh[:], rhs=gat,
                                             start=(i == 0), stop=(i == nt - 1))
                            i += 1

                    aggsb = sbp.tile([128, 128], fp32, tag="aggsb")
                    nc.scalar.copy(out=aggsb[:], in_=agg[:])
                    cnt = sbp.tile([128, 1], fp32, tag="cnt")
                    nc.vector.reduce_sum(out=cnt[:], in_=aggsb[:], axis=AX)
                    logcnt = sbp.tile([128, 1], fp32, tag="logcnt")
                    nc.scalar.activation(out=logcnt[:], in_=cnt[:], func=AF.Ln,
                                         scale=1.0 / G)
                    # QA^T = Qbig^T(lhsT=qbig) @ aggr^T
                    trp = psp.tile([128, 128], fp32, tag="trp", space="PSUM")
                    nc.tensor.transpose(out=trp[:], in_=aggsb[:],
                                        identity=ident_f[:])
                    aggT = sbp.tile([128, 128], fp32, tag="aggT")
                    nc.scalar.copy(out=aggT[:], in_=trp[:])
                    qaT = psp.tile([128, 128], fp32, tag="qa", space="PSUM")
                    nc.tensor.matmul(out=qaT[:], lhsT=qbig[:], rhs=aggT[:],
                                     start=True, stop=True)
                    qaTsb = sbp.tile([128, 128], fp32, tag="qaTsb")
                    nc.scalar.copy(out=qaTsb[:], in_=qaT[:])
                    qa2 = psp.tile([128, 128], fp32, tag="trp", space="PSUM")
                    nc.tensor.transpose(out=qa2[:], in_=qaTsb[:],
                                        identity=ident_f[:])
                    bx = psp.tile([128, 128], fp32, tag="bx", space="PSUM")
                    nc.tensor.matmul(out=bx[:],
                                     lhsT=ohXT[:, b * 128:(b + 1) * 128],
                                     rhs=barrT[:], start=True, stop=True)
                    bxsb = sbp.tile([128, 128], fp32, tag="bxsb")
                    nc.scalar.copy(out=bxsb[:], in_=bx[:])
                    u = sbp.tile([128, 128], fp32, tag="u")
                    nc.vector.tensor_tensor(out=u[:], in0=qa2[:], in1=bxsb[:],
                                            op=OP.mult)
                    Z = sbp.tile([128, G], fp32, tag="Z")
                    nc.vector.reduce_sum(out=Z[:], in_=u[:].rearrange(
                        "p (g c) -> p g c", c=C), axis=AX)
                    logZ = sbp.tile([128, G], fp32, tag="logZ")
                    nc.scalar.activation(out=logZ[:], in_=Z[:], func=AF.Ln)
                    nc.vector.tensor_scalar(
                        out=out_sb[:, b * 64 + l * G:b * 64 + (l + 1) * G],
                        in0=logZ[:], scalar1=logcnt[:, 0:1], scalar2=None,
                        op0=OP.subtract)
                    if l < L - 1:
                        rz = sbp.tile([128, G], fp32, tag="rz")
                        nc.vector.reciprocal(out=rz[:], in_=Z[:])
                        h = sbp.tile([128, 128], bf16, tag="h")
                        nc.vector.tensor_tensor(
                            out=h[:].rearrange("p (g c) -> p g c", c=C),
                            in0=u[:].rearrange("p (g c) -> p g c", c=C),
                            in1=rz[:].to_broadcast([128, G, C]), op=OP.mult)
                        if b < lo_nb:
                            nc.sync.dma_start(
                                out=h_slice_lo[l][b * 128:b * 128 + nn, :],
                                in_=h[:nn, :])
                        else:
                            bo = b - lo_nb
                            nc.sync.dma_start(
                                out=h_slice_hi[l][bo * 128:bo * 128 + nn, :],
                                in_=h[:nn, :])
                        if b == lo_nb - 1:
                            nc.gpsimd.collective_compute(
                                "AllGather", OP.bypass, replica_groups=rgroups,
                                ins=[h_slice_lo[l][:]], outs=[h_full_lo[l][:]])

            # ---- write lls out
            if nb > 1:
                nc.sync.dma_start(
                    out=lls_d[:(nb - 1) * 128, :].rearrange(
                        "(b p) c -> p b c", p=128),
                    in_=out_sb[:].rearrange("p (b c) -> p b c", c=64)[:, :nb - 1, :])
            nc.sync.dma_start(
                out=lls_d[(nb - 1) * 128:, :],
                in_=out_sb[:last_nn, (nb - 1) * 64:nb * 64])

    nc.compile()
    return nc


# ---- entry point ------------------------------------------------------------

def kernel(x, edge_index, lambda_B0, lambda_Pi, lambda_Q, lambda_B):
    cfg = Cfg()
    cores, TA, TB = preprocess(x, edge_index, cfg)
    consts = make_consts()
    nc = build_nc(cfg, TA, TB)

    from concourse.bass_utils import run_bass_kernel_spmd
    params = permute_params(lambda_B0, lambda_Pi, lambda_Q, lambda_B)
    in_maps = []
    for c in range(cfg.ncores):
        m = dict(cores[c])
        m.update(params)
        m.update({k: np.ascontiguousarray(v) for k, v in consts.items()})
        in_maps.append(m)

    res = run_bass_kernel_spmd(nc, in_maps, core_ids=list(range(cfg.ncores)))
    out = np.concatenate([res.results[c]["lls"] for c in range(cfg.ncores)],
                         axis=0)
    return out.reshape(N, L, G).astype(np.float32)
